# revision 41
# baseline (speedup 1.0000x reference)
"""Trainium2 Bass kernel for nn_AttentionBlock (GAT-style attention block).

Sharding: data-parallel over batch (bs=8) across 8 NeuronCores; params
replicated.  Each core computes one batch's full [n, n] attention.

Per-core math (n=2048, c=128, e=64):
  x' = x + enc @ W_enc.T + b_enc
  xn = (x' - mean)/std            (LN1, affine folded into W0/bias)
  h  = xn @ (g1*W0.T) + b1@W0.T
  s1 = h@a1, s2 = h@a2
  v[i,j]  = exp(leaky_relu(s1_i + s2_j))      (2 ACT passes, fused bias)
  Z_i     = sum_j v[i,j]                      (fused accum_out)
  atten   = v * (1/Z)                         (GPSIMD tensor_scalar)
  att     = (v @ h) * (1/Z)                   (PE transpose + bf16 matmuls)
  h_next  = elu(LN2(att + x') @ (g2*W1.T) + b2@W1.T)
"""
import sys
sys.path.insert(0, "/opt/trn_rl_repo")

import numpy as np
from contextlib import ExitStack

import concourse.bass as bass
import concourse.tile as tile
from concourse import mybir, masks
from concourse.tile import add_dep_helper
from concourse.bass_utils import run_bass_kernel_spmd

P = 128          # partitions / c
NT = 16          # n tiles
N = 2048         # n
E = 64           # enc dim
EPS = 1e-5
ALPHA = 0.01
F32 = mybir.dt.float32
BF16 = mybir.dt.bfloat16
FP16 = mybir.dt.float16
AF = mybir.ActivationFunctionType
OP = mybir.AluOpType


# ---------------------------------------------------------------------------
# transitive sem-wait reduction + multi-wait splitting (walrus allows only one
# sync wait per instruction in this toolchain)
# ---------------------------------------------------------------------------
def _reduce_waits(nc):
    import os
    import bass_rust
    prune = bool(os.environ.get("DO_PRUNE"))
    f = nc.m.functions[0]
    eng_know, sem_value, snaps, unknown = {}, {}, {}, set()

    def join(dst, src):
        for s, v in src.items():
            if dst.get(s, 0) < v:
                dst[s] = v

    for blk in f.blocks:
        for inst in blk.instructions:
            si = inst.sync_info
            if si is None:
                continue
            waits, updates = list(si.on_wait), list(si.on_update)
            if not waits and not updates:
                continue
            K = eng_know.setdefault(str(inst.engine), {})
            tname0 = type(inst).__name__
            inst_is_dma = "DMA" in tname0.upper()
            changed = False
            new_waits = []
            for w in waits:
                if (w.wait_reg is not None or w.wait_mode != "sem-ge-imm"
                        or w.sync_type != "semaphore"):
                    new_waits.append(w)
                    continue
                s, v = w.ant_name, w.wait_value
                # DMA-on-DMA completion waits also serialize xbar-mode
                # transitions (DMATranspose vs DMACopy HW deadlock) -- keep.
                if inst_is_dma and s.startswith(("DMAHW", "DMASW")):
                    new_waits.append(w)
                    for val_after, kn in snaps.get(s, ()):
                        if val_after >= v:
                            join(K, kn)
                            break
                    if K.get(s, 0) < v:
                        K[s] = v
                    continue
                if prune and s not in unknown and K.get(s, 0) >= v:
                    changed = True
                    continue
                new_waits.append(w)
                for val_after, kn in snaps.get(s, ()):
                    if val_after >= v:
                        join(K, kn)
                        break
                if K.get(s, 0) < v:
                    K[s] = v
            tname = type(inst).__name__
            is_dma = ("DMA" in tname.upper()) or any(
                u.ant_name.startswith(("DMAHW", "DMASW")) for u in updates
                if u.sync_type == "semaphore")
            for u in updates:
                if u.sync_type != "semaphore":
                    continue
                s = u.ant_name
                if u.update_reg is not None or u.update_mode not in (
                        "sem-inc", "sem-add-imm"):
                    unknown.add(s)
                    continue
                d = 1 if u.update_mode == "sem-inc" else u.update_value
                sem_value[s] = sem_value.get(s, 0) + d
                snap_k = dict(K)
                snap_k[s] = max(snap_k.get(s, 0), sem_value[s])
                snaps.setdefault(s, []).append((sem_value[s], snap_k))
                if not is_dma and K.get(s, 0) < sem_value[s]:
                    K[s] = sem_value[s]
            if changed:
                inst.sync_info = bass_rust.SyncInfo(
                    on_wait=new_waits, on_update=updates)

    for blk in f.blocks:
        il = blk.instructions
        out = []
        for inst in il:
            si = inst.sync_info
            tname = type(inst).__name__
            if (si is not None and len(si.on_wait) >= 2
                    and "Barrier" not in tname):
                waits = list(si.on_wait)
                for k, w in enumerate(waits[:-1]):
                    nop = mybir.InstNoOp(
                        name=f"{inst.name}-wsplit{k}", ins=[], outs=[])
                    nop.engine = inst.engine
                    nop.bass_nofuse = True
                    nop.sync_info = bass_rust.SyncInfo(on_wait=[w], on_update=[])
                    out.append(nop)
                inst.sync_info = bass_rust.SyncInfo(
                    on_wait=[waits[-1]], on_update=list(si.on_update))
            out.append(inst)
        if len(out) != len(il):
            blk.instructions = out


# ---------------------------------------------------------------------------
def _build():
    nc = bass.Bass(trn_type="TRN2")
    dx = nc.dram_tensor("x", (N, P), F32, kind="ExternalInput")
    denc = nc.dram_tensor("enc", (N, E), F32, kind="ExternalInput")
    dWenc = nc.dram_tensor("W_enc", (P, E), F32, kind="ExternalInput")
    dbenc = nc.dram_tensor("b_enc", (P,), F32, kind="ExternalInput")
    dg1 = nc.dram_tensor("g1", (P,), F32, kind="ExternalInput")
    db1 = nc.dram_tensor("b1", (P,), F32, kind="ExternalInput")
    dg2 = nc.dram_tensor("g2", (P,), F32, kind="ExternalInput")
    db2 = nc.dram_tensor("b2", (P,), F32, kind="ExternalInput")
    dW0 = nc.dram_tensor("W0", (P, P), F32, kind="ExternalInput")
    dWa = nc.dram_tensor("Wa", (2 * P,), F32, kind="ExternalInput")
    dW1 = nc.dram_tensor("W1", (P, P), F32, kind="ExternalInput")
    dhn = nc.dram_tensor("h_next", (N, P), F32, kind="ExternalOutput")
    datt = nc.dram_tensor("atten", (N, N), F32, kind="ExternalOutput")

    with ExitStack() as ctx:
        tc = ctx.enter_context(tile.TileContext(nc))
        cpool = ctx.enter_context(tc.tile_pool(name="cpool", bufs=1))
        wide = ctx.enter_context(tc.tile_pool(name="wide", bufs=1))
        work = ctx.enter_context(tc.tile_pool(name="work", bufs=3))
        sml = ctx.enter_context(tc.tile_pool(name="sml", bufs=3))
        big = ctx.enter_context(tc.tile_pool(name="big", bufs=2))
        vpool = ctx.enter_context(tc.tile_pool(name="vpool", bufs=3))
        apool = ctx.enter_context(tc.tile_pool(name="apool", bufs=3))

        # ---- constants / params -------------------------------------------
        with tc.tile_pool(name="ps0a", bufs=2, space="PSUM") as ps0a, \
             tc.tile_pool(name="ps0b", bufs=1, space="PSUM") as ps0b:

            X = wide.tile([P, NT, P], F32)
            x_wide = bass.AP(tensor=dx[:, :].tensor, offset=0,
                             ap=[[P, P], [P * P, NT], [1, P]])
            nc.sync.dma_start(out=X[:], in_=x_wide)

            ident = cpool.tile([P, P], F32)
            masks.make_identity(nc, ident[:])
            ident_bf = cpool.tile([P, P], BF16)
            nc.vector.tensor_copy(ident_bf[:], ident[:])
            ident_h = cpool.tile([P, P], FP16)
            nc.vector.tensor_copy(ident_h[:], ident[:])
            ones_row_h = cpool.tile([1, P], FP16)
            nc.vector.memset(ones_row_h[:], 1.0)
            scrap = ps0b.tile([1, 1], F32, tag="tmp")
            nc.tensor.matmul(scrap[:], ident[:1, :1], ident[:1, :1])

            W0sb = cpool.tile([P, P], F32)
            nc.sync.dma_start(out=W0sb[:], in_=dW0[:, :])
            W1sb = cpool.tile([P, P], F32)
            nc.sync.dma_start(out=W1sb[:], in_=dW1[:, :])
            g1c = cpool.tile([P, 1], F32)
            nc.sync.dma_start(out=g1c[:], in_=dg1[:].rearrange("(p o) -> p o", o=1))
            b1c = cpool.tile([P, 1], F32)
            nc.sync.dma_start(out=b1c[:], in_=db1[:].rearrange("(p o) -> p o", o=1))
            g2c = cpool.tile([P, 1], F32)
            nc.sync.dma_start(out=g2c[:], in_=dg2[:].rearrange("(p o) -> p o", o=1))
            b2c = cpool.tile([P, 1], F32)
            nc.sync.dma_start(out=b2c[:], in_=db2[:].rearrange("(p o) -> p o", o=1))
            a12 = cpool.tile([P, 2], F32)
            wa = dWa[:].rearrange("(k p) -> k p", k=2)
            nc.sync.dma_start(out=a12[:, 0:1], in_=wa[0:1, :].rearrange("o p -> p o"))
            nc.sync.dma_start(out=a12[:, 1:2], in_=wa[1:2, :].rearrange("o p -> p o"))

            enc_lhsT = cpool.tile([E + 1, N], FP16)
            nc.vector.memset(enc_lhsT[E:E + 1, :], 1.0)
            encw = cpool.tile([P, NT, E], F32)
            enc_wide = bass.AP(tensor=denc[:, :].tensor, offset=0,
                               ap=[[E, P], [E * P, NT], [1, E]])
            nc.sync.dma_start(out=encw[:], in_=enc_wide)
            Wenc_sb = cpool.tile([P, E], F32)
            nc.sync.dma_start(out=Wenc_sb[:], in_=dWenc[:, :])
            pe_rhs = cpool.tile([E + 1, P], FP16)
            benc_row = cpool.tile([1, P], F32)
            nc.sync.dma_start(out=benc_row[:],
                              in_=dbenc[:].rearrange("(o p) -> o p", o=1))
            nc.vector.tensor_copy(pe_rhs[E:E + 1, :], benc_row[:])

            ones_row = cpool.tile([1, P], F32)
            nc.vector.memset(ones_row[:], 1.0)
            eps_c = cpool.tile([P, 1], F32)
            nc.vector.memset(eps_c[:], EPS)

            Wenc_h = cpool.tile([P, E], FP16)
            nc.vector.tensor_copy(Wenc_h[:], Wenc_sb[:])
            encw_h = cpool.tile([P, NT, E], FP16)
            nc.vector.tensor_copy(encw_h[:], encw[:])
            wet_ps = ps0b.tile([E, P], FP16, tag="tmph")
            nc.tensor.transpose(wet_ps[:], Wenc_h[:], ident_h[:])
            nc.vector.tensor_copy(pe_rhs[0:E, :], wet_ps[:])
            for _i in range(NT):
                et_ps = ps0a.tile([E, P], FP16, tag="pe")
                nc.tensor.transpose(et_ps[:], encw_h[:, _i, :], ident_h[:])
                nc.vector.tensor_copy(enc_lhsT[0:E, _i * P:(_i + 1) * P],
                                      et_ps[:])

            w0t_ps = ps0b.tile([P, P], F32, tag="tmp")
            nc.tensor.transpose(w0t_ps[:], W0sb[:], ident[:])
            W0T = cpool.tile([P, P], F32)
            nc.vector.tensor_copy(W0T[:], w0t_ps[:])
            W0g = cpool.tile([P, P], F32)
            nc.vector.tensor_scalar_mul(W0g[:], W0T[:], g1c[:])
            w1t_ps = ps0b.tile([P, P], F32, tag="tmp")
            nc.tensor.transpose(w1t_ps[:], W1sb[:], ident[:])
            W1T = cpool.tile([P, P], F32)
            nc.vector.tensor_copy(W1T[:], w1t_ps[:])
            W1g = cpool.tile([P, P], F32)
            nc.vector.tensor_scalar_mul(W1g[:], W1T[:], g2c[:])

            q12_ps = ps0b.tile([P, 2], F32, tag="tmp")
            nc.tensor.matmul(q12_ps[:], W0sb[:], a12[:])
            q12 = cpool.tile([P, 2], F32)
            nc.vector.tensor_scalar_mul(q12[:], q12_ps[:], g1c[:])
            q12h = cpool.tile([P, 2], FP16)
            nc.vector.tensor_copy(q12h[:], q12[:])

            b0r_ps = ps0b.tile([1, P], F32, tag="tmp")
            nc.tensor.matmul(b0r_ps[:], b1c[:], W0T[:])
            bias0_row = cpool.tile([1, P], F32)
            nc.vector.tensor_copy(bias0_row[:], b0r_ps[:])
            b0c_ps = ps0b.tile([P, 1], F32, tag="tmp")
            nc.tensor.matmul(b0c_ps[:], W0T[:], b1c[:])
            bias0_col = cpool.tile([P, 1], F32)
            nc.vector.tensor_copy(bias0_col[:], b0c_ps[:])
            b1r_ps = ps0b.tile([1, P], F32, tag="tmp")
            nc.tensor.matmul(b1r_ps[:], b2c[:], W1T[:])
            bias1_row = cpool.tile([1, P], F32)
            nc.vector.tensor_copy(bias1_row[:], b1r_ps[:])

            c12_ps = ps0b.tile([1, 2], F32, tag="tmp")
            nc.tensor.matmul(c12_ps[:], bias0_col[:], a12[:])
            c12_row = cpool.tile([1, 2], F32)
            nc.vector.tensor_copy(c12_row[:], c12_ps[:])
            c12b_ps = ps0b.tile([P, 2], F32, tag="tmp")
            nc.tensor.matmul(c12b_ps[:], ones_row[:], c12_row[:])
            c12_tmp = cpool.tile([P, 2], F32)
            nc.vector.tensor_copy(c12_tmp[:], c12b_ps[:])
            const12 = cpool.tile([P, 2], F32)
            nc.gpsimd.tensor_copy(const12[:], c12_tmp[:])

            # ---- phase 0: pos-enc, LN1, h, s1/s2 --------------------------

            XNT = wide.tile([P, NT, P], FP16)
            Hbf = wide.tile([P, NT, P], BF16)
            S1 = wide.tile([P, NT], F32)
            s2_row = cpool.tile([1, N], F32)

            for i in range(NT):
                pe_ps = ps0a.tile([P, P], F32, tag="pe")
                nc.tensor.matmul(pe_ps[:], enc_lhsT[:, i * P:(i + 1) * P],
                                 pe_rhs[:])
                nc.vector.tensor_tensor(out=X[:, i, :], in0=X[:, i, :],
                                        in1=pe_ps[:], op=OP.add)
                bs = sml.tile([P, 6], F32, tag="bs")
                nc.vector.bn_stats(out=bs[:], in_=X[:, i, :])
                mv = sml.tile([P, 2], F32, tag="mv")
                nc.vector.bn_aggr(out=mv[:], in_=bs[:])
                stdv = sml.tile([P, 1], F32, tag="stdv")
                nc.scalar.activation(stdv[:], mv[:, 1:2], AF.Sqrt, bias=eps_c[:])
                rstd = sml.tile([P, 1], F32, tag="rstd")
                nc.vector.reciprocal(rstd[:], stdv[:])
                ms = sml.tile([P, 2], F32, tag="ms")
                nc.gpsimd.tensor_copy(ms[:, 0:1], mv[:, 0:1])
                nc.gpsimd.tensor_copy(ms[:, 1:2], rstd[:])

                xn = work.tile([P, P], FP16, tag="xn")
                nc.vector.tensor_scalar(xn[:], X[:, i, :], ms[:, 0:1],
                                        ms[:, 1:2], OP.subtract, OP.mult)
                xnt_ps = ps0b.tile([P, P], FP16, tag="xnt")
                nc.tensor.transpose(xnt_ps[:], xn[:], ident_h[:])
                nc.vector.tensor_copy(XNT[:, i, :], xnt_ps[:])

                s12_ps = ps0b.tile([P, 2], F32, tag="s12")
                nc.tensor.matmul(s12_ps[:], XNT[:, i, :], q12h[:])
                nc.vector.tensor_scalar(S1[:, i:i + 1], s12_ps[:, 0:1],
                                        const12[:, 0:1], None, OP.add)
                s2c = sml.tile([P, 1], F32, tag="s2c")
                nc.vector.tensor_scalar(s2c[:], s12_ps[:, 1:2],
                                        const12[:, 1:2], None, OP.add)
                s2r_ps = ps0b.tile([1, P], F32, tag="s2r")
                nc.tensor.matmul(s2r_ps[:], s2c[:], ident[:])
                nc.vector.tensor_copy(s2_row[:, i * P:(i + 1) * P], s2r_ps[:])

            s2b = wide.tile([P, N], F32)
            for k in range(4):
                s2b_ps = ps0b.tile([P, 512], F32, tag="xnt")
                nc.tensor.matmul(s2b_ps[:], ones_row[:],
                                 s2_row[:, k * 512:(k + 1) * 512])
                nc.vector.tensor_copy(s2b[:, k * 512:(k + 1) * 512], s2b_ps[:])

            # h tiles here (bf16): overlap the ACT-bound phase 1
            W0gbf = cpool.tile([P, P], FP16)
            nc.vector.tensor_copy(W0gbf[:], W0g[:])
            bias0_row_bf = cpool.tile([1, P], FP16)
            nc.vector.tensor_copy(bias0_row_bf[:], bias0_row[:])
            W1gbf = cpool.tile([P, P], FP16)
            nc.vector.tensor_copy(W1gbf[:], W1g[:])
            bias1_row_bf = cpool.tile([1, P], FP16)
            nc.vector.tensor_copy(bias1_row_bf[:], bias1_row[:])
            for i in range(NT):
                h_ps = ps0b.tile([P, P], F32, tag="h")
                nc.tensor.matmul(h_ps[:], XNT[:, i, :], W0gbf[:],
                                 start=True, stop=False)
                nc.tensor.matmul(h_ps[:], ones_row_h[:], bias0_row_bf[:],
                                 start=False, stop=True)
                nc.vector.tensor_copy(Hbf[:, i, :], h_ps[:])

        # ---- phase 1: big attention loop ----------------------------------
        ATTN = wide.tile([P, NT, P], F32)
        RES = X
        MV2 = wide.tile([P, NT, 2], F32)
        mv2_t = []
        last_exp = None
        u_insts, v_insts = [], []
        with tc.tile_pool(name="ps1", bufs=2, space="PSUM") as ps1, \
             tc.tile_pool(name="upool", bufs=7) as upool:
            for i in range(NT):
                u = upool.tile([P, N], F32, tag="u")
                ui = nc.scalar.activation(u[:], s2b[:], AF.Lrelu,
                                          bias=S1[:, i:i + 1], scale=1.0,
                                          alpha=ALPHA)
                u_insts.append(ui)
                v = vpool.tile([P, N], F32, tag="v")
                zc = sml.tile([P, 1], F32, tag="zc")
                last_exp = nc.scalar.activation(v[:], u[:], AF.Exp, bias=0.0,
                                                scale=1.0, accum_out=zc[:])
                v_insts.append(last_exp)
                rz = sml.tile([P, 1], F32, tag="rz")
                nc.vector.reciprocal(rz[:], zc[:])
                rzp = sml.tile([P, 1], F32, tag="rzp")
                nc.gpsimd.tensor_copy(rzp[:], rz[:])

                at = apool.tile([P, N], F32, tag="at")
                nc.vector.tensor_scalar(at[:], v[:], rzp[:], None, OP.mult)
                nc.sync.dma_start(out=datt[i * P:(i + 1) * P, :], in_=at[:])

                vbf = big.tile([P, N], BF16, tag="vbf")
                nc.vector.tensor_copy(vbf[:], v[:])
                vt = big.tile([P, NT, P], BF16, tag="vt")
                for half in range(2):
                    vt_ps = ps1.tile([P, 1024], BF16, tag="vtp")
                    for jj in range(8):
                        j = half * 8 + jj
                        nc.tensor.transpose(vt_ps[:, jj * P:(jj + 1) * P],
                                            vbf[:, j * P:(j + 1) * P],
                                            ident_bf[:])
                    nc.vector.tensor_copy(
                        vt[:, half * 8:(half + 1) * 8, :], vt_ps[:])

                att_ps = ps1.tile([P, P], F32, tag="att")
                for j in range(NT):
                    nc.tensor.matmul(att_ps[:], vt[:, j, :],
                                     Hbf[:, j, :], start=(j == 0),
                                     stop=(j == NT - 1))
                nc.vector.tensor_scalar(ATTN[:, i, :], att_ps[:], rzp[:], None,
                                        OP.mult)
                nc.vector.tensor_tensor(out=X[:, i, :], in0=ATTN[:, i, :],
                                        in1=X[:, i, :], op=OP.add)
                bs2 = sml.tile([P, 6], F32, tag="bs2")
                nc.vector.bn_stats(out=bs2[:], in_=RES[:, i, :])
                mv2i = sml.tile([P, 2], F32, tag=f"mv2_{i}")
                nc.vector.bn_aggr(out=mv2i[:], in_=bs2[:])
                mv2_t.append(mv2i)

        # deterministic ACT order: groups of 4 lrelus then 4 exps (one
        # table-set switch per group boundary instead of per op)
        GK = 4
        act_order = []
        for g in range(0, NT, GK):
            act_order.extend(u_insts[g:g + GK])
            act_order.extend(v_insts[g:g + GK])
        for a, b in zip(act_order[1:], act_order[:-1]):
            add_dep_helper(a.ins, b.ins, sync=False, reason="ACT table grouping")

        # ---- phase 2: batched LN2 rsqrt, W1, ELU --------------------------
        with tc.tile_pool(name="ps2", bufs=2, space="PSUM") as ps2:
            T2 = wide.tile([P, NT, P], F32)
            first_sqrt2 = None
            prev_sqrt2 = None
            for i in range(NT):
                stdv2 = sml.tile([P, 1], F32, tag="stdv2")
                inst = nc.scalar.activation(stdv2[:], mv2_t[i][:, 1:2], AF.Sqrt,
                                            bias=eps_c[:])
                if prev_sqrt2 is not None:
                    add_dep_helper(inst.ins, prev_sqrt2.ins, sync=False,
                                   reason="sqrt chain")
                prev_sqrt2 = inst
                if first_sqrt2 is None:
                    first_sqrt2 = inst
                rstd2 = sml.tile([P, 1], F32, tag="rstd2")
                nc.vector.reciprocal(rstd2[:], stdv2[:])
                ms2 = sml.tile([P, 2], F32, tag="ms2")
                nc.gpsimd.tensor_copy(ms2[:, 0:1], mv2_t[i][:, 0:1])
                nc.gpsimd.tensor_copy(ms2[:, 1:2], rstd2[:])
                xn2 = work.tile([P, P], FP16, tag="xn2")
                nc.vector.tensor_scalar(xn2[:], RES[:, i, :], ms2[:, 0:1],
                                        ms2[:, 1:2], OP.subtract, OP.mult)
                xn2t_ps = ps2.tile([P, P], FP16, tag="xn2t")
                nc.tensor.transpose(xn2t_ps[:], xn2[:], ident_h[:])
                xn2t = work.tile([P, P], FP16, tag="xn2ts")
                nc.vector.tensor_copy(xn2t[:], xn2t_ps[:])
                hn_ps = ps2.tile([P, P], F32, tag="hn")
                nc.tensor.matmul(hn_ps[:], xn2t[:], W1gbf[:],
                                 start=True, stop=False)
                nc.tensor.matmul(hn_ps[:], ones_row_h[:], bias1_row_bf[:],
                                 start=False, stop=True)
                nc.vector.tensor_copy(T2[:, i, :], hn_ps[:])

            if first_sqrt2 is not None and last_exp is not None:
                add_dep_helper(first_sqrt2.ins, last_exp.ins, sync=False,
                               reason="group sqrt table-set after exp set")

            # batched ELU: elu(x) = max(x,0) + min(exp(x)-1, 0)
            E1 = wide.tile([P, NT, P], F32)
            elu_exp = nc.scalar.activation(E1[:], T2[:], AF.Exp, bias=0.0,
                                           scale=1.0)
            add_dep_helper(elu_exp.ins, first_sqrt2.ins, sync=False,
                           reason="exp set after sqrt set")
            nc.vector.tensor_scalar(E1[:], E1[:], -1.0, 0.0, OP.add, OP.min)
            nc.vector.tensor_scalar(T2[:], T2[:], 0.0, None, OP.max)
            nc.vector.tensor_tensor(out=E1[:], in0=E1[:], in1=T2[:], op=OP.add)
            HN = E1
            hn_wide = bass.AP(tensor=dhn[:, :].tensor, offset=0,
                              ap=[[P, P], [P * P, NT], [1, P]])
            nc.sync.dma_start(out=hn_wide, in_=HN[:])

    _reduce_waits(nc)
    return nc


_NC_CACHE = None


def _get_nc():
    global _NC_CACHE
    if _NC_CACHE is None:
        _NC_CACHE = _build()
    return _NC_CACHE


def kernel(x, enc, W_enc, b_enc, g1, b1, g2, b2, W0, Wa, W1, **_ignored):
    x = np.asarray(x, dtype=np.float32)
    enc = np.asarray(enc, dtype=np.float32)
    params = {
        "W_enc": np.ascontiguousarray(np.asarray(W_enc, np.float32)),
        "b_enc": np.ascontiguousarray(np.asarray(b_enc, np.float32)),
        "g1": np.ascontiguousarray(np.asarray(g1, np.float32)),
        "b1": np.ascontiguousarray(np.asarray(b1, np.float32)),
        "g2": np.ascontiguousarray(np.asarray(g2, np.float32)),
        "b2": np.ascontiguousarray(np.asarray(b2, np.float32)),
        "W0": np.ascontiguousarray(np.asarray(W0, np.float32)),
        "Wa": np.ascontiguousarray(np.asarray(Wa, np.float32)),
        "W1": np.ascontiguousarray(np.asarray(W1, np.float32)),
    }
    bs = x.shape[0]
    nc = _get_nc()
    in_maps = []
    for b in range(bs):
        m = {"x": np.ascontiguousarray(x[b]),
             "enc": np.ascontiguousarray(enc[b])}
        m.update(params)
        in_maps.append(m)
    res = run_bass_kernel_spmd(nc, in_maps, list(range(bs)))
    h_next = np.stack([res.results[b]["h_next"] for b in range(bs)])
    atten = np.stack([res.results[b]["atten"] for b in range(bs)])
    return h_next, atten


# revision 42
# speedup vs baseline: 1.0155x; 1.0155x over previous
"""Trainium2 Bass kernel for nn_AttentionBlock (GAT-style attention block).

Sharding: data-parallel over batch (bs=8) across 8 NeuronCores; params
replicated.  Each core computes one batch's full [n, n] attention.

Per-core math (n=2048, c=128, e=64):
  x' = x + enc @ W_enc.T + b_enc
  xn = (x' - mean)/std            (LN1, affine folded into W0/bias)
  h  = xn @ (g1*W0.T) + b1@W0.T
  s1 = h@a1, s2 = h@a2
  v[i,j]  = exp(leaky_relu(s1_i + s2_j))      (2 ACT passes, fused bias)
  Z_i     = sum_j v[i,j]                      (fused accum_out)
  atten   = v * (1/Z)                         (GPSIMD tensor_scalar)
  att     = (v @ h) * (1/Z)                   (PE transpose + bf16 matmuls)
  h_next  = elu(LN2(att + x') @ (g2*W1.T) + b2@W1.T)
"""
import sys
sys.path.insert(0, "/opt/trn_rl_repo")

import numpy as np
from contextlib import ExitStack

import concourse.bass as bass
import concourse.tile as tile
from concourse import mybir, masks
from concourse.tile import add_dep_helper
from concourse.bass_utils import run_bass_kernel_spmd

P = 128          # partitions / c
NT = 16          # n tiles
N = 2048         # n
E = 64           # enc dim
EPS = 1e-5
ALPHA = 0.01
F32 = mybir.dt.float32
BF16 = mybir.dt.bfloat16
FP16 = mybir.dt.float16
AF = mybir.ActivationFunctionType
OP = mybir.AluOpType


# ---------------------------------------------------------------------------
# transitive sem-wait reduction + multi-wait splitting (walrus allows only one
# sync wait per instruction in this toolchain)
# ---------------------------------------------------------------------------
def _reduce_waits(nc):
    import os
    import bass_rust
    prune = bool(os.environ.get("DO_PRUNE"))
    f = nc.m.functions[0]
    eng_know, sem_value, snaps, unknown = {}, {}, {}, set()

    def join(dst, src):
        for s, v in src.items():
            if dst.get(s, 0) < v:
                dst[s] = v

    for blk in f.blocks:
        for inst in blk.instructions:
            si = inst.sync_info
            if si is None:
                continue
            waits, updates = list(si.on_wait), list(si.on_update)
            if not waits and not updates:
                continue
            K = eng_know.setdefault(str(inst.engine), {})
            tname0 = type(inst).__name__
            inst_is_dma = "DMA" in tname0.upper()
            changed = False
            new_waits = []
            for w in waits:
                if (w.wait_reg is not None or w.wait_mode != "sem-ge-imm"
                        or w.sync_type != "semaphore"):
                    new_waits.append(w)
                    continue
                s, v = w.ant_name, w.wait_value
                # DMA-on-DMA completion waits also serialize xbar-mode
                # transitions (DMATranspose vs DMACopy HW deadlock) -- keep.
                if inst_is_dma and s.startswith(("DMAHW", "DMASW")):
                    new_waits.append(w)
                    for val_after, kn in snaps.get(s, ()):
                        if val_after >= v:
                            join(K, kn)
                            break
                    if K.get(s, 0) < v:
                        K[s] = v
                    continue
                if prune and s not in unknown and K.get(s, 0) >= v:
                    changed = True
                    continue
                new_waits.append(w)
                for val_after, kn in snaps.get(s, ()):
                    if val_after >= v:
                        join(K, kn)
                        break
                if K.get(s, 0) < v:
                    K[s] = v
            tname = type(inst).__name__
            is_dma = ("DMA" in tname.upper()) or any(
                u.ant_name.startswith(("DMAHW", "DMASW")) for u in updates
                if u.sync_type == "semaphore")
            for u in updates:
                if u.sync_type != "semaphore":
                    continue
                s = u.ant_name
                if u.update_reg is not None or u.update_mode not in (
                        "sem-inc", "sem-add-imm"):
                    unknown.add(s)
                    continue
                d = 1 if u.update_mode == "sem-inc" else u.update_value
                sem_value[s] = sem_value.get(s, 0) + d
                snap_k = dict(K)
                snap_k[s] = max(snap_k.get(s, 0), sem_value[s])
                snaps.setdefault(s, []).append((sem_value[s], snap_k))
                if not is_dma and K.get(s, 0) < sem_value[s]:
                    K[s] = sem_value[s]
            if changed:
                inst.sync_info = bass_rust.SyncInfo(
                    on_wait=new_waits, on_update=updates)

    for blk in f.blocks:
        il = blk.instructions
        out = []
        for inst in il:
            si = inst.sync_info
            tname = type(inst).__name__
            if (si is not None and len(si.on_wait) >= 2
                    and "Barrier" not in tname):
                waits = list(si.on_wait)
                for k, w in enumerate(waits[:-1]):
                    nop = mybir.InstNoOp(
                        name=f"{inst.name}-wsplit{k}", ins=[], outs=[])
                    nop.engine = inst.engine
                    nop.bass_nofuse = True
                    nop.sync_info = bass_rust.SyncInfo(on_wait=[w], on_update=[])
                    out.append(nop)
                inst.sync_info = bass_rust.SyncInfo(
                    on_wait=[waits[-1]], on_update=list(si.on_update))
            out.append(inst)
        if len(out) != len(il):
            blk.instructions = out


# ---------------------------------------------------------------------------
def _build():
    nc = bass.Bass(trn_type="TRN2")
    dx = nc.dram_tensor("x", (N, P), F32, kind="ExternalInput")
    denc = nc.dram_tensor("enc", (N, E), F32, kind="ExternalInput")
    dWenc = nc.dram_tensor("W_enc", (P, E), F32, kind="ExternalInput")
    dbenc = nc.dram_tensor("b_enc", (P,), F32, kind="ExternalInput")
    dg1 = nc.dram_tensor("g1", (P,), F32, kind="ExternalInput")
    db1 = nc.dram_tensor("b1", (P,), F32, kind="ExternalInput")
    dg2 = nc.dram_tensor("g2", (P,), F32, kind="ExternalInput")
    db2 = nc.dram_tensor("b2", (P,), F32, kind="ExternalInput")
    dW0 = nc.dram_tensor("W0", (P, P), F32, kind="ExternalInput")
    dWa = nc.dram_tensor("Wa", (2 * P,), F32, kind="ExternalInput")
    dW1 = nc.dram_tensor("W1", (P, P), F32, kind="ExternalInput")
    dhn = nc.dram_tensor("h_next", (N, P), F32, kind="ExternalOutput")
    datt = nc.dram_tensor("atten", (N, N), F32, kind="ExternalOutput")

    with ExitStack() as ctx:
        tc = ctx.enter_context(tile.TileContext(nc))
        cpool = ctx.enter_context(tc.tile_pool(name="cpool", bufs=1))
        wide = ctx.enter_context(tc.tile_pool(name="wide", bufs=1))
        work = ctx.enter_context(tc.tile_pool(name="work", bufs=3))
        sml = ctx.enter_context(tc.tile_pool(name="sml", bufs=3))
        big = ctx.enter_context(tc.tile_pool(name="big", bufs=2))
        vpool = ctx.enter_context(tc.tile_pool(name="vpool", bufs=3))
        apool = ctx.enter_context(tc.tile_pool(name="apool", bufs=3))

        # ---- constants / params -------------------------------------------
        with tc.tile_pool(name="ps0a", bufs=2, space="PSUM") as ps0a, \
             tc.tile_pool(name="ps0b", bufs=1, space="PSUM") as ps0b:

            X = wide.tile([P, NT, P], F32)
            x_wide = bass.AP(tensor=dx[:, :].tensor, offset=0,
                             ap=[[P, P], [P * P, NT], [1, P]])
            nc.sync.dma_start(out=X[:], in_=x_wide)

            ident = cpool.tile([P, P], F32)
            masks.make_identity(nc, ident[:])
            ident_bf = cpool.tile([P, P], BF16)
            nc.vector.tensor_copy(ident_bf[:], ident[:])
            ident_h = cpool.tile([P, P], FP16)
            nc.vector.tensor_copy(ident_h[:], ident[:])
            ones_row_h = cpool.tile([1, P], FP16)
            nc.vector.memset(ones_row_h[:], 1.0)
            scrap = ps0b.tile([1, 1], F32, tag="tmp")
            nc.tensor.matmul(scrap[:], ident[:1, :1], ident[:1, :1])

            W0sb = cpool.tile([P, P], F32)
            nc.sync.dma_start(out=W0sb[:], in_=dW0[:, :])
            W1sb = cpool.tile([P, P], F32)
            nc.sync.dma_start(out=W1sb[:], in_=dW1[:, :])
            g1c = cpool.tile([P, 1], F32)
            nc.sync.dma_start(out=g1c[:], in_=dg1[:].rearrange("(p o) -> p o", o=1))
            b1c = cpool.tile([P, 1], F32)
            nc.sync.dma_start(out=b1c[:], in_=db1[:].rearrange("(p o) -> p o", o=1))
            g2c = cpool.tile([P, 1], F32)
            nc.sync.dma_start(out=g2c[:], in_=dg2[:].rearrange("(p o) -> p o", o=1))
            b2c = cpool.tile([P, 1], F32)
            nc.sync.dma_start(out=b2c[:], in_=db2[:].rearrange("(p o) -> p o", o=1))
            a12 = cpool.tile([P, 2], F32)
            wa = dWa[:].rearrange("(k p) -> k p", k=2)
            nc.sync.dma_start(out=a12[:, 0:1], in_=wa[0:1, :].rearrange("o p -> p o"))
            nc.sync.dma_start(out=a12[:, 1:2], in_=wa[1:2, :].rearrange("o p -> p o"))

            enc_lhsT = cpool.tile([E + 1, N], FP16)
            nc.vector.memset(enc_lhsT[E:E + 1, :], 1.0)
            encw = cpool.tile([P, NT, E], F32)
            enc_wide = bass.AP(tensor=denc[:, :].tensor, offset=0,
                               ap=[[E, P], [E * P, NT], [1, E]])
            nc.sync.dma_start(out=encw[:], in_=enc_wide)
            Wenc_sb = cpool.tile([P, E], F32)
            nc.sync.dma_start(out=Wenc_sb[:], in_=dWenc[:, :])
            pe_rhs = cpool.tile([E + 1, P], FP16)
            benc_row = cpool.tile([1, P], F32)
            nc.sync.dma_start(out=benc_row[:],
                              in_=dbenc[:].rearrange("(o p) -> o p", o=1))
            nc.vector.tensor_copy(pe_rhs[E:E + 1, :], benc_row[:])

            ones_row = cpool.tile([1, P], F32)
            nc.vector.memset(ones_row[:], 1.0)
            eps_c = cpool.tile([P, 1], F32)
            nc.vector.memset(eps_c[:], EPS)

            Wenc_h = cpool.tile([P, E], FP16)
            nc.vector.tensor_copy(Wenc_h[:], Wenc_sb[:])
            encw_h = cpool.tile([P, NT, E], FP16)
            nc.vector.tensor_copy(encw_h[:], encw[:])
            wet_ps = ps0b.tile([E, P], FP16, tag="tmph")
            nc.tensor.transpose(wet_ps[:], Wenc_h[:], ident_h[:])
            nc.vector.tensor_copy(pe_rhs[0:E, :], wet_ps[:])
            for _i in range(NT):
                et_ps = ps0a.tile([E, P], FP16, tag="pe")
                nc.tensor.transpose(et_ps[:], encw_h[:, _i, :], ident_h[:])
                nc.vector.tensor_copy(enc_lhsT[0:E, _i * P:(_i + 1) * P],
                                      et_ps[:])

            w0t_ps = ps0b.tile([P, P], F32, tag="tmp")
            nc.tensor.transpose(w0t_ps[:], W0sb[:], ident[:])
            W0T = cpool.tile([P, P], F32)
            nc.vector.tensor_copy(W0T[:], w0t_ps[:])
            W0g = cpool.tile([P, P], F32)
            nc.vector.tensor_scalar_mul(W0g[:], W0T[:], g1c[:])
            w1t_ps = ps0b.tile([P, P], F32, tag="tmp")
            nc.tensor.transpose(w1t_ps[:], W1sb[:], ident[:])
            W1T = cpool.tile([P, P], F32)
            nc.vector.tensor_copy(W1T[:], w1t_ps[:])
            W1g = cpool.tile([P, P], F32)
            nc.vector.tensor_scalar_mul(W1g[:], W1T[:], g2c[:])

            q12_ps = ps0b.tile([P, 2], F32, tag="tmp")
            nc.tensor.matmul(q12_ps[:], W0sb[:], a12[:])
            q12 = cpool.tile([P, 2], F32)
            nc.vector.tensor_scalar_mul(q12[:], q12_ps[:], g1c[:])
            q12h = cpool.tile([P, 2], FP16)
            nc.vector.tensor_copy(q12h[:], q12[:])

            b0r_ps = ps0b.tile([1, P], F32, tag="tmp")
            nc.tensor.matmul(b0r_ps[:], b1c[:], W0T[:])
            bias0_row = cpool.tile([1, P], F32)
            nc.vector.tensor_copy(bias0_row[:], b0r_ps[:])
            b0c_ps = ps0b.tile([P, 1], F32, tag="tmp")
            nc.tensor.matmul(b0c_ps[:], W0T[:], b1c[:])
            bias0_col = cpool.tile([P, 1], F32)
            nc.vector.tensor_copy(bias0_col[:], b0c_ps[:])
            b1r_ps = ps0b.tile([1, P], F32, tag="tmp")
            nc.tensor.matmul(b1r_ps[:], b2c[:], W1T[:])
            bias1_row = cpool.tile([1, P], F32)
            nc.vector.tensor_copy(bias1_row[:], b1r_ps[:])

            c12_ps = ps0b.tile([1, 2], F32, tag="tmp")
            nc.tensor.matmul(c12_ps[:], bias0_col[:], a12[:])
            c12_row = cpool.tile([1, 2], F32)
            nc.vector.tensor_copy(c12_row[:], c12_ps[:])
            c12b_ps = ps0b.tile([P, 2], F32, tag="tmp")
            nc.tensor.matmul(c12b_ps[:], ones_row[:], c12_row[:])
            c12_tmp = cpool.tile([P, 2], F32)
            nc.vector.tensor_copy(c12_tmp[:], c12b_ps[:])
            const12 = cpool.tile([P, 2], F32)
            nc.gpsimd.tensor_copy(const12[:], c12_tmp[:])

            # ---- phase 0: pos-enc, LN1, h, s1/s2 --------------------------

            XNT = wide.tile([P, NT, P], FP16)
            Hbf = wide.tile([P, NT, P], FP16)
            S1 = wide.tile([P, NT], F32)
            s2_row = cpool.tile([1, N], F32)

            for i in range(NT):
                pe_ps = ps0a.tile([P, P], F32, tag="pe")
                nc.tensor.matmul(pe_ps[:], enc_lhsT[:, i * P:(i + 1) * P],
                                 pe_rhs[:])
                nc.vector.tensor_tensor(out=X[:, i, :], in0=X[:, i, :],
                                        in1=pe_ps[:], op=OP.add)
                bs = sml.tile([P, 6], F32, tag="bs")
                nc.vector.bn_stats(out=bs[:], in_=X[:, i, :])
                mv = sml.tile([P, 2], F32, tag="mv")
                nc.vector.bn_aggr(out=mv[:], in_=bs[:])
                stdv = sml.tile([P, 1], F32, tag="stdv")
                nc.scalar.activation(stdv[:], mv[:, 1:2], AF.Sqrt, bias=eps_c[:])
                rstd = sml.tile([P, 1], F32, tag="rstd")
                nc.vector.reciprocal(rstd[:], stdv[:])
                ms = sml.tile([P, 2], F32, tag="ms")
                nc.gpsimd.tensor_copy(ms[:, 0:1], mv[:, 0:1])
                nc.gpsimd.tensor_copy(ms[:, 1:2], rstd[:])

                xn = work.tile([P, P], FP16, tag="xn")
                nc.vector.tensor_scalar(xn[:], X[:, i, :], ms[:, 0:1],
                                        ms[:, 1:2], OP.subtract, OP.mult)
                xnt_ps = ps0b.tile([P, P], FP16, tag="xnt")
                nc.tensor.transpose(xnt_ps[:], xn[:], ident_h[:])
                nc.vector.tensor_copy(XNT[:, i, :], xnt_ps[:])

                s12_ps = ps0b.tile([P, 2], F32, tag="s12")
                nc.tensor.matmul(s12_ps[:], XNT[:, i, :], q12h[:])
                nc.vector.tensor_scalar(S1[:, i:i + 1], s12_ps[:, 0:1],
                                        const12[:, 0:1], None, OP.add)
                s2c = sml.tile([P, 1], F32, tag="s2c")
                nc.vector.tensor_scalar(s2c[:], s12_ps[:, 1:2],
                                        const12[:, 1:2], None, OP.add)
                s2r_ps = ps0b.tile([1, P], F32, tag="s2r")
                nc.tensor.matmul(s2r_ps[:], s2c[:], ident[:])
                nc.vector.tensor_copy(s2_row[:, i * P:(i + 1) * P], s2r_ps[:])

            s2b = wide.tile([P, N], F32)
            for k in range(4):
                s2b_ps = ps0b.tile([P, 512], F32, tag="xnt")
                nc.tensor.matmul(s2b_ps[:], ones_row[:],
                                 s2_row[:, k * 512:(k + 1) * 512])
                nc.vector.tensor_copy(s2b[:, k * 512:(k + 1) * 512], s2b_ps[:])

            # h tiles here (bf16): overlap the ACT-bound phase 1
            W0gbf = cpool.tile([P, P], FP16)
            nc.vector.tensor_copy(W0gbf[:], W0g[:])
            bias0_row_bf = cpool.tile([1, P], FP16)
            nc.vector.tensor_copy(bias0_row_bf[:], bias0_row[:])
            W1gbf = cpool.tile([P, P], FP16)
            nc.vector.tensor_copy(W1gbf[:], W1g[:])
            bias1_row_bf = cpool.tile([1, P], FP16)
            nc.vector.tensor_copy(bias1_row_bf[:], bias1_row[:])
            for i in range(NT):
                h_ps = ps0b.tile([P, P], F32, tag="h")
                nc.tensor.matmul(h_ps[:], XNT[:, i, :], W0gbf[:],
                                 start=True, stop=False)
                nc.tensor.matmul(h_ps[:], ones_row_h[:], bias0_row_bf[:],
                                 start=False, stop=True)
                nc.vector.tensor_copy(Hbf[:, i, :], h_ps[:])

        # ---- phase 1: big attention loop ----------------------------------
        ATTN = wide.tile([P, NT, P], F32)
        RES = X
        MV2 = wide.tile([P, NT, 2], F32)
        mv2_t = []
        last_exp = None
        u_insts, v_insts = [], []
        with tc.tile_pool(name="ps1", bufs=2, space="PSUM") as ps1, \
             tc.tile_pool(name="upool", bufs=7) as upool:
            for i in range(NT):
                u = upool.tile([P, N], F32, tag="u")
                ui = nc.scalar.activation(u[:], s2b[:], AF.Lrelu,
                                          bias=S1[:, i:i + 1], scale=1.0,
                                          alpha=ALPHA)
                u_insts.append(ui)
                v = vpool.tile([P, N], FP16, tag="v")
                zc = sml.tile([P, 1], F32, tag="zc")
                last_exp = nc.scalar.activation(v[:], u[:], AF.Exp, bias=0.0,
                                                scale=1.0, accum_out=zc[:])
                v_insts.append(last_exp)
                rz = sml.tile([P, 1], F32, tag="rz")
                nc.vector.reciprocal(rz[:], zc[:])
                rzp = sml.tile([P, 1], F32, tag="rzp")
                nc.gpsimd.tensor_copy(rzp[:], rz[:])

                at = apool.tile([P, N], F32, tag="at")
                nc.vector.tensor_scalar(at[:], v[:], rzp[:], None, OP.mult)
                nc.sync.dma_start(out=datt[i * P:(i + 1) * P, :], in_=at[:])

                vt = big.tile([P, NT, P], FP16, tag="vt")
                for half in range(2):
                    vt_ps = ps1.tile([P, 1024], FP16, tag="vtp")
                    for jj in range(8):
                        j = half * 8 + jj
                        nc.tensor.transpose(vt_ps[:, jj * P:(jj + 1) * P],
                                            v[:, j * P:(j + 1) * P],
                                            ident_h[:])
                    nc.vector.tensor_copy(
                        vt[:, half * 8:(half + 1) * 8, :], vt_ps[:])

                att_ps = ps1.tile([P, P], F32, tag="att")
                for j in range(NT):
                    nc.tensor.matmul(att_ps[:], vt[:, j, :],
                                     Hbf[:, j, :], start=(j == 0),
                                     stop=(j == NT - 1))
                nc.vector.tensor_scalar(ATTN[:, i, :], att_ps[:], rzp[:], None,
                                        OP.mult)
                nc.vector.tensor_tensor(out=X[:, i, :], in0=ATTN[:, i, :],
                                        in1=X[:, i, :], op=OP.add)
                bs2 = sml.tile([P, 6], F32, tag="bs2")
                nc.vector.bn_stats(out=bs2[:], in_=RES[:, i, :])
                mv2i = sml.tile([P, 2], F32, tag=f"mv2_{i}")
                nc.vector.bn_aggr(out=mv2i[:], in_=bs2[:])
                mv2_t.append(mv2i)

        # deterministic ACT order: groups of 4 lrelus then 4 exps (one
        # table-set switch per group boundary instead of per op)
        GK = 4
        act_order = []
        for g in range(0, NT, GK):
            act_order.extend(u_insts[g:g + GK])
            act_order.extend(v_insts[g:g + GK])
        for a, b in zip(act_order[1:], act_order[:-1]):
            add_dep_helper(a.ins, b.ins, sync=False, reason="ACT table grouping")

        # ---- phase 2: batched LN2 rsqrt, W1, ELU --------------------------
        with tc.tile_pool(name="ps2", bufs=2, space="PSUM") as ps2:
            T2 = wide.tile([P, NT, P], F32)
            first_sqrt2 = None
            prev_sqrt2 = None
            for i in range(NT):
                stdv2 = sml.tile([P, 1], F32, tag="stdv2")
                inst = nc.scalar.activation(stdv2[:], mv2_t[i][:, 1:2], AF.Sqrt,
                                            bias=eps_c[:])
                if prev_sqrt2 is not None:
                    add_dep_helper(inst.ins, prev_sqrt2.ins, sync=False,
                                   reason="sqrt chain")
                prev_sqrt2 = inst
                if first_sqrt2 is None:
                    first_sqrt2 = inst
                rstd2 = sml.tile([P, 1], F32, tag="rstd2")
                nc.vector.reciprocal(rstd2[:], stdv2[:])
                ms2 = sml.tile([P, 2], F32, tag="ms2")
                nc.gpsimd.tensor_copy(ms2[:, 0:1], mv2_t[i][:, 0:1])
                nc.gpsimd.tensor_copy(ms2[:, 1:2], rstd2[:])
                xn2 = work.tile([P, P], FP16, tag="xn2")
                nc.vector.tensor_scalar(xn2[:], RES[:, i, :], ms2[:, 0:1],
                                        ms2[:, 1:2], OP.subtract, OP.mult)
                xn2t_ps = ps2.tile([P, P], FP16, tag="xn2t")
                nc.tensor.transpose(xn2t_ps[:], xn2[:], ident_h[:])
                xn2t = work.tile([P, P], FP16, tag="xn2ts")
                nc.vector.tensor_copy(xn2t[:], xn2t_ps[:])
                hn_ps = ps2.tile([P, P], F32, tag="hn")
                nc.tensor.matmul(hn_ps[:], xn2t[:], W1gbf[:],
                                 start=True, stop=False)
                nc.tensor.matmul(hn_ps[:], ones_row_h[:], bias1_row_bf[:],
                                 start=False, stop=True)
                nc.vector.tensor_copy(T2[:, i, :], hn_ps[:])

            if first_sqrt2 is not None and last_exp is not None:
                add_dep_helper(first_sqrt2.ins, last_exp.ins, sync=False,
                               reason="group sqrt table-set after exp set")

            # batched ELU: elu(x) = max(x,0) + min(exp(x)-1, 0)
            E1 = wide.tile([P, NT, P], F32)
            elu_exp = nc.scalar.activation(E1[:], T2[:], AF.Exp, bias=0.0,
                                           scale=1.0)
            add_dep_helper(elu_exp.ins, first_sqrt2.ins, sync=False,
                           reason="exp set after sqrt set")
            nc.vector.tensor_scalar(E1[:], E1[:], -1.0, 0.0, OP.add, OP.min)
            nc.vector.tensor_scalar(T2[:], T2[:], 0.0, None, OP.max)
            nc.vector.tensor_tensor(out=E1[:], in0=E1[:], in1=T2[:], op=OP.add)
            HN = E1
            hn_wide = bass.AP(tensor=dhn[:, :].tensor, offset=0,
                              ap=[[P, P], [P * P, NT], [1, P]])
            nc.sync.dma_start(out=hn_wide, in_=HN[:])

    _reduce_waits(nc)
    return nc


_NC_CACHE = None


def _get_nc():
    global _NC_CACHE
    if _NC_CACHE is None:
        _NC_CACHE = _build()
    return _NC_CACHE


def kernel(x, enc, W_enc, b_enc, g1, b1, g2, b2, W0, Wa, W1, **_ignored):
    x = np.asarray(x, dtype=np.float32)
    enc = np.asarray(enc, dtype=np.float32)
    params = {
        "W_enc": np.ascontiguousarray(np.asarray(W_enc, np.float32)),
        "b_enc": np.ascontiguousarray(np.asarray(b_enc, np.float32)),
        "g1": np.ascontiguousarray(np.asarray(g1, np.float32)),
        "b1": np.ascontiguousarray(np.asarray(b1, np.float32)),
        "g2": np.ascontiguousarray(np.asarray(g2, np.float32)),
        "b2": np.ascontiguousarray(np.asarray(b2, np.float32)),
        "W0": np.ascontiguousarray(np.asarray(W0, np.float32)),
        "Wa": np.ascontiguousarray(np.asarray(Wa, np.float32)),
        "W1": np.ascontiguousarray(np.asarray(W1, np.float32)),
    }
    bs = x.shape[0]
    nc = _get_nc()
    in_maps = []
    for b in range(bs):
        m = {"x": np.ascontiguousarray(x[b]),
             "enc": np.ascontiguousarray(enc[b])}
        m.update(params)
        in_maps.append(m)
    res = run_bass_kernel_spmd(nc, in_maps, list(range(bs)))
    h_next = np.stack([res.results[b]["h_next"] for b in range(bs)])
    atten = np.stack([res.results[b]["atten"] for b in range(bs)])
    return h_next, atten


# revision 43
# speedup vs baseline: 1.0292x; 1.0135x over previous
"""Trainium2 Bass kernel for nn_AttentionBlock (GAT-style attention block).

Sharding: data-parallel over batch (bs=8) across 8 NeuronCores; params
replicated.  Each core computes one batch's full [n, n] attention.

Per-core math (n=2048, c=128, e=64):
  x' = x + enc @ W_enc.T + b_enc
  xn = (x' - mean)/std            (LN1, affine folded into W0/bias)
  h  = xn @ (g1*W0.T) + b1@W0.T
  s1 = h@a1, s2 = h@a2
  v[i,j]  = exp(leaky_relu(s1_i + s2_j))      (2 ACT passes, fused bias)
  Z_i     = sum_j v[i,j]                      (fused accum_out)
  atten   = v * (1/Z)                         (GPSIMD tensor_scalar)
  att     = (v @ h) * (1/Z)                   (PE transpose + bf16 matmuls)
  h_next  = elu(LN2(att + x') @ (g2*W1.T) + b2@W1.T)
"""
import sys
sys.path.insert(0, "/opt/trn_rl_repo")

import numpy as np
from contextlib import ExitStack

import concourse.bass as bass
import concourse.tile as tile
from concourse import mybir, masks
from concourse.tile import add_dep_helper
from concourse.bass_utils import run_bass_kernel_spmd

P = 128          # partitions / c
NT = 16          # n tiles
N = 2048         # n
E = 64           # enc dim
EPS = 1e-5
ALPHA = 0.01
F32 = mybir.dt.float32
BF16 = mybir.dt.bfloat16
FP16 = mybir.dt.float16
AF = mybir.ActivationFunctionType
OP = mybir.AluOpType


# ---------------------------------------------------------------------------
# transitive sem-wait reduction + multi-wait splitting (walrus allows only one
# sync wait per instruction in this toolchain)
# ---------------------------------------------------------------------------
def _reduce_waits(nc):
    import os
    import bass_rust
    prune = bool(os.environ.get("DO_PRUNE"))
    f = nc.m.functions[0]
    eng_know, sem_value, snaps, unknown = {}, {}, {}, set()

    def join(dst, src):
        for s, v in src.items():
            if dst.get(s, 0) < v:
                dst[s] = v

    for blk in f.blocks:
        for inst in blk.instructions:
            si = inst.sync_info
            if si is None:
                continue
            waits, updates = list(si.on_wait), list(si.on_update)
            if not waits and not updates:
                continue
            K = eng_know.setdefault(str(inst.engine), {})
            tname0 = type(inst).__name__
            inst_is_dma = "DMA" in tname0.upper()
            changed = False
            new_waits = []
            for w in waits:
                if (w.wait_reg is not None or w.wait_mode != "sem-ge-imm"
                        or w.sync_type != "semaphore"):
                    new_waits.append(w)
                    continue
                s, v = w.ant_name, w.wait_value
                # DMA-on-DMA completion waits also serialize xbar-mode
                # transitions (DMATranspose vs DMACopy HW deadlock) -- keep.
                if inst_is_dma and s.startswith(("DMAHW", "DMASW")):
                    new_waits.append(w)
                    for val_after, kn in snaps.get(s, ()):
                        if val_after >= v:
                            join(K, kn)
                            break
                    if K.get(s, 0) < v:
                        K[s] = v
                    continue
                if prune and s not in unknown and K.get(s, 0) >= v:
                    changed = True
                    continue
                new_waits.append(w)
                for val_after, kn in snaps.get(s, ()):
                    if val_after >= v:
                        join(K, kn)
                        break
                if K.get(s, 0) < v:
                    K[s] = v
            tname = type(inst).__name__
            is_dma = ("DMA" in tname.upper()) or any(
                u.ant_name.startswith(("DMAHW", "DMASW")) for u in updates
                if u.sync_type == "semaphore")
            for u in updates:
                if u.sync_type != "semaphore":
                    continue
                s = u.ant_name
                if u.update_reg is not None or u.update_mode not in (
                        "sem-inc", "sem-add-imm"):
                    unknown.add(s)
                    continue
                d = 1 if u.update_mode == "sem-inc" else u.update_value
                sem_value[s] = sem_value.get(s, 0) + d
                snap_k = dict(K)
                snap_k[s] = max(snap_k.get(s, 0), sem_value[s])
                snaps.setdefault(s, []).append((sem_value[s], snap_k))
                if not is_dma and K.get(s, 0) < sem_value[s]:
                    K[s] = sem_value[s]
            if changed:
                inst.sync_info = bass_rust.SyncInfo(
                    on_wait=new_waits, on_update=updates)

    for blk in f.blocks:
        il = blk.instructions
        out = []
        for inst in il:
            si = inst.sync_info
            tname = type(inst).__name__
            if (si is not None and len(si.on_wait) >= 2
                    and "Barrier" not in tname):
                waits = list(si.on_wait)
                for k, w in enumerate(waits[:-1]):
                    nop = mybir.InstNoOp(
                        name=f"{inst.name}-wsplit{k}", ins=[], outs=[])
                    nop.engine = inst.engine
                    nop.bass_nofuse = True
                    nop.sync_info = bass_rust.SyncInfo(on_wait=[w], on_update=[])
                    out.append(nop)
                inst.sync_info = bass_rust.SyncInfo(
                    on_wait=[waits[-1]], on_update=list(si.on_update))
            out.append(inst)
        if len(out) != len(il):
            blk.instructions = out


# ---------------------------------------------------------------------------
def _build():
    nc = bass.Bass(trn_type="TRN2")
    dx = nc.dram_tensor("x", (N, P), F32, kind="ExternalInput")
    denc = nc.dram_tensor("enc", (N, E), F32, kind="ExternalInput")
    dWenc = nc.dram_tensor("W_enc", (P, E), F32, kind="ExternalInput")
    dbenc = nc.dram_tensor("b_enc", (P,), F32, kind="ExternalInput")
    dg1 = nc.dram_tensor("g1", (P,), F32, kind="ExternalInput")
    db1 = nc.dram_tensor("b1", (P,), F32, kind="ExternalInput")
    dg2 = nc.dram_tensor("g2", (P,), F32, kind="ExternalInput")
    db2 = nc.dram_tensor("b2", (P,), F32, kind="ExternalInput")
    dW0 = nc.dram_tensor("W0", (P, P), F32, kind="ExternalInput")
    dWa = nc.dram_tensor("Wa", (2 * P,), F32, kind="ExternalInput")
    dW1 = nc.dram_tensor("W1", (P, P), F32, kind="ExternalInput")
    dhn = nc.dram_tensor("h_next", (N, P), F32, kind="ExternalOutput")
    datt = nc.dram_tensor("atten", (N, N), F32, kind="ExternalOutput")

    with ExitStack() as ctx:
        tc = ctx.enter_context(tile.TileContext(nc))
        cpool = ctx.enter_context(tc.tile_pool(name="cpool", bufs=1))
        wide = ctx.enter_context(tc.tile_pool(name="wide", bufs=1))
        work = ctx.enter_context(tc.tile_pool(name="work", bufs=3))
        sml = ctx.enter_context(tc.tile_pool(name="sml", bufs=3))
        big = ctx.enter_context(tc.tile_pool(name="big", bufs=3))
        vpool = ctx.enter_context(tc.tile_pool(name="vpool", bufs=4))
        apool = ctx.enter_context(tc.tile_pool(name="apool", bufs=3))

        # ---- constants / params -------------------------------------------
        with tc.tile_pool(name="ps0a", bufs=2, space="PSUM") as ps0a, \
             tc.tile_pool(name="ps0b", bufs=1, space="PSUM") as ps0b:

            X = wide.tile([P, NT, P], F32)
            x_wide = bass.AP(tensor=dx[:, :].tensor, offset=0,
                             ap=[[P, P], [P * P, NT], [1, P]])
            nc.sync.dma_start(out=X[:], in_=x_wide)

            ident = cpool.tile([P, P], F32)
            masks.make_identity(nc, ident[:])
            ident_bf = cpool.tile([P, P], BF16)
            nc.vector.tensor_copy(ident_bf[:], ident[:])
            ident_h = cpool.tile([P, P], FP16)
            nc.vector.tensor_copy(ident_h[:], ident[:])
            ones_row_h = cpool.tile([1, P], FP16)
            nc.vector.memset(ones_row_h[:], 1.0)
            scrap = ps0b.tile([1, 1], F32, tag="tmp")
            nc.tensor.matmul(scrap[:], ident[:1, :1], ident[:1, :1])

            W0sb = cpool.tile([P, P], F32)
            nc.sync.dma_start(out=W0sb[:], in_=dW0[:, :])
            W1sb = cpool.tile([P, P], F32)
            nc.sync.dma_start(out=W1sb[:], in_=dW1[:, :])
            g1c = cpool.tile([P, 1], F32)
            nc.sync.dma_start(out=g1c[:], in_=dg1[:].rearrange("(p o) -> p o", o=1))
            b1c = cpool.tile([P, 1], F32)
            nc.sync.dma_start(out=b1c[:], in_=db1[:].rearrange("(p o) -> p o", o=1))
            g2c = cpool.tile([P, 1], F32)
            nc.sync.dma_start(out=g2c[:], in_=dg2[:].rearrange("(p o) -> p o", o=1))
            b2c = cpool.tile([P, 1], F32)
            nc.sync.dma_start(out=b2c[:], in_=db2[:].rearrange("(p o) -> p o", o=1))
            a12 = cpool.tile([P, 2], F32)
            wa = dWa[:].rearrange("(k p) -> k p", k=2)
            nc.sync.dma_start(out=a12[:, 0:1], in_=wa[0:1, :].rearrange("o p -> p o"))
            nc.sync.dma_start(out=a12[:, 1:2], in_=wa[1:2, :].rearrange("o p -> p o"))

            enc_lhsT = cpool.tile([E + 1, N], FP16)
            nc.vector.memset(enc_lhsT[E:E + 1, :], 1.0)
            encw = cpool.tile([P, NT, E], F32)
            enc_wide = bass.AP(tensor=denc[:, :].tensor, offset=0,
                               ap=[[E, P], [E * P, NT], [1, E]])
            nc.sync.dma_start(out=encw[:], in_=enc_wide)
            Wenc_sb = cpool.tile([P, E], F32)
            nc.sync.dma_start(out=Wenc_sb[:], in_=dWenc[:, :])
            pe_rhs = cpool.tile([E + 1, P], FP16)
            benc_row = cpool.tile([1, P], F32)
            nc.sync.dma_start(out=benc_row[:],
                              in_=dbenc[:].rearrange("(o p) -> o p", o=1))
            nc.vector.tensor_copy(pe_rhs[E:E + 1, :], benc_row[:])

            ones_row = cpool.tile([1, P], F32)
            nc.vector.memset(ones_row[:], 1.0)
            eps_c = cpool.tile([P, 1], F32)
            nc.vector.memset(eps_c[:], EPS)

            Wenc_h = cpool.tile([P, E], FP16)
            nc.vector.tensor_copy(Wenc_h[:], Wenc_sb[:])
            encw_h = cpool.tile([P, NT, E], FP16)
            nc.vector.tensor_copy(encw_h[:], encw[:])
            wet_ps = ps0b.tile([E, P], FP16, tag="tmph")
            nc.tensor.transpose(wet_ps[:], Wenc_h[:], ident_h[:])
            nc.vector.tensor_copy(pe_rhs[0:E, :], wet_ps[:])
            for _i in range(NT):
                et_ps = ps0a.tile([E, P], FP16, tag="pe")
                nc.tensor.transpose(et_ps[:], encw_h[:, _i, :], ident_h[:])
                nc.vector.tensor_copy(enc_lhsT[0:E, _i * P:(_i + 1) * P],
                                      et_ps[:])

            w0t_ps = ps0b.tile([P, P], F32, tag="tmp")
            nc.tensor.transpose(w0t_ps[:], W0sb[:], ident[:])
            W0T = cpool.tile([P, P], F32)
            nc.vector.tensor_copy(W0T[:], w0t_ps[:])
            W0g = cpool.tile([P, P], F32)
            nc.vector.tensor_scalar_mul(W0g[:], W0T[:], g1c[:])
            w1t_ps = ps0b.tile([P, P], F32, tag="tmp")
            nc.tensor.transpose(w1t_ps[:], W1sb[:], ident[:])
            W1T = cpool.tile([P, P], F32)
            nc.vector.tensor_copy(W1T[:], w1t_ps[:])
            W1g = cpool.tile([P, P], F32)
            nc.vector.tensor_scalar_mul(W1g[:], W1T[:], g2c[:])

            q12_ps = ps0b.tile([P, 2], F32, tag="tmp")
            nc.tensor.matmul(q12_ps[:], W0sb[:], a12[:])
            q12 = cpool.tile([P, 2], F32)
            nc.vector.tensor_scalar_mul(q12[:], q12_ps[:], g1c[:])
            q12h = cpool.tile([P, 2], FP16)
            nc.vector.tensor_copy(q12h[:], q12[:])

            b0r_ps = ps0b.tile([1, P], F32, tag="tmp")
            nc.tensor.matmul(b0r_ps[:], b1c[:], W0T[:])
            bias0_row = cpool.tile([1, P], F32)
            nc.vector.tensor_copy(bias0_row[:], b0r_ps[:])
            b0c_ps = ps0b.tile([P, 1], F32, tag="tmp")
            nc.tensor.matmul(b0c_ps[:], W0T[:], b1c[:])
            bias0_col = cpool.tile([P, 1], F32)
            nc.vector.tensor_copy(bias0_col[:], b0c_ps[:])
            b1r_ps = ps0b.tile([1, P], F32, tag="tmp")
            nc.tensor.matmul(b1r_ps[:], b2c[:], W1T[:])
            bias1_row = cpool.tile([1, P], F32)
            nc.vector.tensor_copy(bias1_row[:], b1r_ps[:])

            c12_ps = ps0b.tile([1, 2], F32, tag="tmp")
            nc.tensor.matmul(c12_ps[:], bias0_col[:], a12[:])
            c12_row = cpool.tile([1, 2], F32)
            nc.vector.tensor_copy(c12_row[:], c12_ps[:])
            c12b_ps = ps0b.tile([P, 2], F32, tag="tmp")
            nc.tensor.matmul(c12b_ps[:], ones_row[:], c12_row[:])
            c12_tmp = cpool.tile([P, 2], F32)
            nc.vector.tensor_copy(c12_tmp[:], c12b_ps[:])
            const12 = cpool.tile([P, 2], F32)
            nc.gpsimd.tensor_copy(const12[:], c12_tmp[:])

            # ---- phase 0: pos-enc, LN1, h, s1/s2 --------------------------

            XNT = wide.tile([P, NT, P], FP16)
            Hbf = wide.tile([P, NT, P], FP16)
            S1 = wide.tile([P, NT], F32)
            s2_row = cpool.tile([1, N], F32)

            for i in range(NT):
                pe_ps = ps0a.tile([P, P], F32, tag="pe")
                nc.tensor.matmul(pe_ps[:], enc_lhsT[:, i * P:(i + 1) * P],
                                 pe_rhs[:])
                nc.vector.tensor_tensor(out=X[:, i, :], in0=X[:, i, :],
                                        in1=pe_ps[:], op=OP.add)
                bs = sml.tile([P, 6], F32, tag="bs")
                nc.vector.bn_stats(out=bs[:], in_=X[:, i, :])
                mv = sml.tile([P, 2], F32, tag="mv")
                nc.vector.bn_aggr(out=mv[:], in_=bs[:])
                stdv = sml.tile([P, 1], F32, tag="stdv")
                nc.scalar.activation(stdv[:], mv[:, 1:2], AF.Sqrt, bias=eps_c[:])
                rstd = sml.tile([P, 1], F32, tag="rstd")
                nc.vector.reciprocal(rstd[:], stdv[:])
                ms = sml.tile([P, 2], F32, tag="ms")
                nc.gpsimd.tensor_copy(ms[:, 0:1], mv[:, 0:1])
                nc.gpsimd.tensor_copy(ms[:, 1:2], rstd[:])

                xn = work.tile([P, P], FP16, tag="xn")
                nc.vector.tensor_scalar(xn[:], X[:, i, :], ms[:, 0:1],
                                        ms[:, 1:2], OP.subtract, OP.mult)
                xnt_ps = ps0b.tile([P, P], FP16, tag="xnt")
                nc.tensor.transpose(xnt_ps[:], xn[:], ident_h[:])
                nc.vector.tensor_copy(XNT[:, i, :], xnt_ps[:])

                s12_ps = ps0b.tile([P, 2], F32, tag="s12")
                nc.tensor.matmul(s12_ps[:], XNT[:, i, :], q12h[:])
                nc.vector.tensor_scalar(S1[:, i:i + 1], s12_ps[:, 0:1],
                                        const12[:, 0:1], None, OP.add)
                s2c = sml.tile([P, 1], F32, tag="s2c")
                nc.vector.tensor_scalar(s2c[:], s12_ps[:, 1:2],
                                        const12[:, 1:2], None, OP.add)
                s2r_ps = ps0b.tile([1, P], F32, tag="s2r")
                nc.tensor.matmul(s2r_ps[:], s2c[:], ident[:])
                nc.vector.tensor_copy(s2_row[:, i * P:(i + 1) * P], s2r_ps[:])

            s2b = wide.tile([P, N], F32)
            for k in range(4):
                s2b_ps = ps0b.tile([P, 512], F32, tag="xnt")
                nc.tensor.matmul(s2b_ps[:], ones_row[:],
                                 s2_row[:, k * 512:(k + 1) * 512])
                nc.vector.tensor_copy(s2b[:, k * 512:(k + 1) * 512], s2b_ps[:])

            # h tiles here (bf16): overlap the ACT-bound phase 1
            W0gbf = cpool.tile([P, P], FP16)
            nc.vector.tensor_copy(W0gbf[:], W0g[:])
            bias0_row_bf = cpool.tile([1, P], FP16)
            nc.vector.tensor_copy(bias0_row_bf[:], bias0_row[:])
            W1gbf = cpool.tile([P, P], FP16)
            nc.vector.tensor_copy(W1gbf[:], W1g[:])
            bias1_row_bf = cpool.tile([1, P], FP16)
            nc.vector.tensor_copy(bias1_row_bf[:], bias1_row[:])
            for i in range(NT):
                h_ps = ps0b.tile([P, P], F32, tag="h")
                nc.tensor.matmul(h_ps[:], XNT[:, i, :], W0gbf[:],
                                 start=True, stop=False)
                nc.tensor.matmul(h_ps[:], ones_row_h[:], bias0_row_bf[:],
                                 start=False, stop=True)
                nc.vector.tensor_copy(Hbf[:, i, :], h_ps[:])

        # ---- phase 1: big attention loop ----------------------------------
        ATTN = wide.tile([P, NT, P], F32)
        RES = X
        MV2 = wide.tile([P, NT, 2], F32)
        mv2_t = []
        last_exp = None
        u_insts, v_insts = [], []
        with tc.tile_pool(name="ps1", bufs=2, space="PSUM") as ps1, \
             tc.tile_pool(name="upool", bufs=7) as upool:
            for i in range(NT):
                u = upool.tile([P, N], F32, tag="u")
                ui = nc.scalar.activation(u[:], s2b[:], AF.Lrelu,
                                          bias=S1[:, i:i + 1], scale=1.0,
                                          alpha=ALPHA)
                u_insts.append(ui)
                v = vpool.tile([P, N], FP16, tag="v")
                zc = sml.tile([P, 1], F32, tag="zc")
                last_exp = nc.scalar.activation(v[:], u[:], AF.Exp, bias=0.0,
                                                scale=1.0, accum_out=zc[:])
                v_insts.append(last_exp)
                rz = sml.tile([P, 1], F32, tag="rz")
                nc.vector.reciprocal(rz[:], zc[:])
                rzp = sml.tile([P, 1], F32, tag="rzp")
                nc.gpsimd.tensor_copy(rzp[:], rz[:])

                at = apool.tile([P, N], F32, tag="at")
                nc.vector.tensor_scalar(at[:], v[:], rzp[:], None, OP.mult)
                nc.sync.dma_start(out=datt[i * P:(i + 1) * P, :], in_=at[:])

                vt = big.tile([P, NT, P], FP16, tag="vt")
                for half in range(2):
                    vt_ps = ps1.tile([P, 1024], FP16, tag="vtp")
                    for jj in range(8):
                        j = half * 8 + jj
                        nc.tensor.transpose(vt_ps[:, jj * P:(jj + 1) * P],
                                            v[:, j * P:(j + 1) * P],
                                            ident_h[:])
                    nc.vector.tensor_copy(
                        vt[:, half * 8:(half + 1) * 8, :], vt_ps[:])

                att_ps = ps1.tile([P, P], F32, tag="att")
                for j in range(NT):
                    nc.tensor.matmul(att_ps[:], vt[:, j, :],
                                     Hbf[:, j, :], start=(j == 0),
                                     stop=(j == NT - 1))
                nc.vector.tensor_scalar(ATTN[:, i, :], att_ps[:], rzp[:], None,
                                        OP.mult)
                nc.vector.tensor_tensor(out=X[:, i, :], in0=ATTN[:, i, :],
                                        in1=X[:, i, :], op=OP.add)
                bs2 = sml.tile([P, 6], F32, tag="bs2")
                nc.vector.bn_stats(out=bs2[:], in_=RES[:, i, :])
                mv2i = sml.tile([P, 2], F32, tag=f"mv2_{i}")
                nc.vector.bn_aggr(out=mv2i[:], in_=bs2[:])
                mv2_t.append(mv2i)

        # deterministic ACT order: groups of 4 lrelus then 4 exps (one
        # table-set switch per group boundary instead of per op)
        GK = 4
        act_order = []
        for g in range(0, NT, GK):
            act_order.extend(u_insts[g:g + GK])
            act_order.extend(v_insts[g:g + GK])
        for a, b in zip(act_order[1:], act_order[:-1]):
            add_dep_helper(a.ins, b.ins, sync=False, reason="ACT table grouping")

        # ---- phase 2: batched LN2 rsqrt, W1, ELU --------------------------
        with tc.tile_pool(name="ps2", bufs=2, space="PSUM") as ps2:
            T2 = wide.tile([P, NT, P], F32)
            first_sqrt2 = None
            prev_sqrt2 = None
            for i in range(NT):
                stdv2 = sml.tile([P, 1], F32, tag="stdv2")
                inst = nc.scalar.activation(stdv2[:], mv2_t[i][:, 1:2], AF.Sqrt,
                                            bias=eps_c[:])
                if prev_sqrt2 is not None:
                    add_dep_helper(inst.ins, prev_sqrt2.ins, sync=False,
                                   reason="sqrt chain")
                prev_sqrt2 = inst
                if first_sqrt2 is None:
                    first_sqrt2 = inst
                rstd2 = sml.tile([P, 1], F32, tag="rstd2")
                nc.vector.reciprocal(rstd2[:], stdv2[:])
                ms2 = sml.tile([P, 2], F32, tag="ms2")
                nc.gpsimd.tensor_copy(ms2[:, 0:1], mv2_t[i][:, 0:1])
                nc.gpsimd.tensor_copy(ms2[:, 1:2], rstd2[:])
                xn2 = work.tile([P, P], FP16, tag="xn2")
                nc.vector.tensor_scalar(xn2[:], RES[:, i, :], ms2[:, 0:1],
                                        ms2[:, 1:2], OP.subtract, OP.mult)
                xn2t_ps = ps2.tile([P, P], FP16, tag="xn2t")
                nc.tensor.transpose(xn2t_ps[:], xn2[:], ident_h[:])
                xn2t = work.tile([P, P], FP16, tag="xn2ts")
                nc.vector.tensor_copy(xn2t[:], xn2t_ps[:])
                hn_ps = ps2.tile([P, P], F32, tag="hn")
                nc.tensor.matmul(hn_ps[:], xn2t[:], W1gbf[:],
                                 start=True, stop=False)
                nc.tensor.matmul(hn_ps[:], ones_row_h[:], bias1_row_bf[:],
                                 start=False, stop=True)
                nc.vector.tensor_copy(T2[:, i, :], hn_ps[:])

            if first_sqrt2 is not None and last_exp is not None:
                add_dep_helper(first_sqrt2.ins, last_exp.ins, sync=False,
                               reason="group sqrt table-set after exp set")

            # batched ELU: elu(x) = max(x,0) + min(exp(x)-1, 0)
            E1 = wide.tile([P, NT, P], F32)
            elu_exp = nc.scalar.activation(E1[:], T2[:], AF.Exp, bias=0.0,
                                           scale=1.0)
            add_dep_helper(elu_exp.ins, first_sqrt2.ins, sync=False,
                           reason="exp set after sqrt set")
            nc.vector.tensor_scalar(E1[:], E1[:], -1.0, 0.0, OP.add, OP.min)
            nc.vector.tensor_scalar(T2[:], T2[:], 0.0, None, OP.max)
            nc.vector.tensor_tensor(out=E1[:], in0=E1[:], in1=T2[:], op=OP.add)
            HN = E1
            hn_wide = bass.AP(tensor=dhn[:, :].tensor, offset=0,
                              ap=[[P, P], [P * P, NT], [1, P]])
            nc.sync.dma_start(out=hn_wide, in_=HN[:])

    _reduce_waits(nc)
    return nc


_NC_CACHE = None


def _get_nc():
    global _NC_CACHE
    if _NC_CACHE is None:
        _NC_CACHE = _build()
    return _NC_CACHE


def kernel(x, enc, W_enc, b_enc, g1, b1, g2, b2, W0, Wa, W1, **_ignored):
    x = np.asarray(x, dtype=np.float32)
    enc = np.asarray(enc, dtype=np.float32)
    params = {
        "W_enc": np.ascontiguousarray(np.asarray(W_enc, np.float32)),
        "b_enc": np.ascontiguousarray(np.asarray(b_enc, np.float32)),
        "g1": np.ascontiguousarray(np.asarray(g1, np.float32)),
        "b1": np.ascontiguousarray(np.asarray(b1, np.float32)),
        "g2": np.ascontiguousarray(np.asarray(g2, np.float32)),
        "b2": np.ascontiguousarray(np.asarray(b2, np.float32)),
        "W0": np.ascontiguousarray(np.asarray(W0, np.float32)),
        "Wa": np.ascontiguousarray(np.asarray(Wa, np.float32)),
        "W1": np.ascontiguousarray(np.asarray(W1, np.float32)),
    }
    bs = x.shape[0]
    nc = _get_nc()
    in_maps = []
    for b in range(bs):
        m = {"x": np.ascontiguousarray(x[b]),
             "enc": np.ascontiguousarray(enc[b])}
        m.update(params)
        in_maps.append(m)
    res = run_bass_kernel_spmd(nc, in_maps, list(range(bs)))
    h_next = np.stack([res.results[b]["h_next"] for b in range(bs)])
    atten = np.stack([res.results[b]["atten"] for b in range(bs)])
    return h_next, atten
